# revision 16
# baseline (speedup 1.0000x reference)
"""ECE (expected calibration error) kernel for Trainium2, 8 NeuronCores.

Math: per_bin = |avg_conf - avg_acc| * counts/N  ==  |sum_conf - sum_acc| / N
(when counts>0; both sides 0 when counts==0), so

    ECE = (1/(N*C)) * sum_{b,c} | sum_conf[b,c] - sum_acc[b,c] |

The device computes the heavy O(N*C) part per core (data-parallel over N):
  - V[c]     = sum_n conf[n,c]        (softmax column sums, PE-accumulated)
  - s[n]     = sum_c exp(logits[n,c]) (unshifted; logits bounded, no overflow)
  - max_e[n] = max_c exp(logits[n,c]) (so host can flag rows near bin edges)
The host assembles the per-(bin,class) sums from these:
  - bin 0 holds every element with conf <= 1/15; V gives its sum_conf column
    totals directly.  Rows whose max confidence max_e/s can reach 1/15 are
    recomputed exactly on host (a handful of rows) and their >1/15 elements
    are moved from bin 0 into their true bins.
  - sum_acc needs only conf[n, labels[n]] = exp(logits[n,labels[n]]) / s[n].

Device layout: quad-row tiles [128, 4, 1000] where partition r holds DRAM
rows 4r..4r+3 of the 512-row block -> 16KB contiguous per partition per DMA
descriptor; one 4000-element ACT exp op per tile; bf16 4x-mode Vector
reductions; per-row 1/s folded into the PE column-sum as the stationary.
"""

import os
import sys

import numpy as np

if "/opt/trn_rl_repo" not in sys.path:  # harness may run from a bare dir
    sys.path.insert(0, "/opt/trn_rl_repo")

import concourse.bass as bass
import concourse.tile as tile
from concourse import bacc, mybir
from concourse.bass_utils import run_bass_kernel_spmd

N, C, NB = 65536, 1000, 15
N_CORES = 8
N_LOC = N // N_CORES  # 8192
P = 128
J = 4  # rows per partition per tile
ROWS_PER_TILE = P * J  # 512
T = N_LOC // ROWS_PER_TILE  # 16 tiles per core
NCOL = T * J  # 64 stat columns
F32 = mybir.dt.float32
BF16 = mybir.dt.bfloat16

_CACHE: dict = {}
LAST_RESULT = None  # BassKernelResults of the most recent run (for profiling)


def _build():
    nc = bacc.Bacc("TRN2", target_bir_lowering=False, debug=False, num_devices=N_CORES)

    logits_ext = nc.declare_dram_parameter("logits", [N_LOC, C], F32, isOutput=False)
    v_ext = nc.declare_dram_parameter("v_out", [1, C], F32, isOutput=True)
    s_ext = nc.declare_dram_parameter("s_out", [P, NCOL], F32, isOutput=True)

    NA = 512  # first PSUM bank width
    NB_ = C - NA  # second

    with tile.TileContext(nc) as tc:
        with (
            tc.tile_pool(name="xin", bufs=4) as x_pool,
            tc.tile_pool(name="ework", bufs=3) as e_pool,
            tc.tile_pool(name="small", bufs=4) as w_pool,
            tc.tile_pool(name="accum", bufs=1) as acc_pool,
            tc.tile_pool(name="psum", bufs=1, space="PSUM") as psum_pool,
        ):
            s_acc = acc_pool.tile([P, NCOL], F32)
            pA = psum_pool.tile([1, NA], F32)
            pB = psum_pool.tile([1, NB_], F32)

            # The first and last 512-row blocks run as four J=1 sub-units so
            # the pipeline ramps up after a 512KB DMA (not 2MB) and the tail
            # chain after the final DMA is short.  Middle blocks are quads.
            # Work items: (dram_row_start, n_rows_per_partition, s_col_start)
            work = []
            for k in range(J):
                work.append((k * P, 1, k))
            for t in range(1, T - 1):
                work.append((t * ROWS_PER_TILE, J, t * J))
            for k in range(J):
                work.append(((T - 1) * ROWS_PER_TILE + k * P, 1, (T - 1) * J + k))

            n_items = len(work)
            for it, (row0, jj, col0) in enumerate(work):
                x = x_pool.tile([P, jj, C], F32, tag=f"x{jj}")
                src = logits_ext[row0 : row0 + P * jj, :].rearrange(
                    "(p j) c -> p j c", j=jj
                )
                nc.sync.dma_start(out=x[:], in_=src)

                e = e_pool.tile([P, jj, C], BF16, tag=f"e{jj}")
                nc.scalar.activation(e[:], x[:], mybir.ActivationFunctionType.Exp)

                nc.vector.tensor_reduce(
                    s_acc[:, col0 : col0 + jj],
                    e[:],
                    axis=mybir.AxisListType.X,
                    op=mybir.AluOpType.add,
                )

                w32 = w_pool.tile([P, jj], F32, tag=f"w32{jj}")
                nc.vector.reciprocal(w32[:], s_acc[:, col0 : col0 + jj])
                w16 = w_pool.tile([P, jj], BF16, tag=f"w16{jj}")
                nc.vector.tensor_copy(w16[:], w32[:])

                for j in range(jj):
                    first = it == 0 and j == 0
                    last = it == n_items - 1 and j == jj - 1
                    nc.tensor.matmul(
                        pA[:], w16[:, j : j + 1], e[:, j, :NA], start=first, stop=last
                    )
                    nc.tensor.matmul(
                        pB[:], w16[:, j : j + 1], e[:, j, NA:], start=first, stop=last
                    )

            vout = acc_pool.tile([1, C], F32)
            nc.vector.tensor_copy(vout[:, :NA], pA[:])
            nc.vector.tensor_copy(vout[:, NA:], pB[:])
            nc.sync.dma_start(out=v_ext[:], in_=vout[:])
            nc.sync.dma_start(out=s_ext[:], in_=s_acc[:])

    nc.compile()
    return nc


def _get_nc():
    if "nc" not in _CACHE:
        _CACHE["nc"] = _build()
    return _CACHE["nc"]


def _unscramble(a: np.ndarray) -> np.ndarray:
    # Middle blocks (quad layout): a[r, t*J + j] holds row t*512 + r*J + j.
    # First/last blocks ran as four J=1 units: a[r, t*J + k] holds row
    # t*512 + k*128 + r.
    out = np.empty(N_LOC, dtype=a.dtype)
    out[:ROWS_PER_TILE] = a[:, :J].T.reshape(ROWS_PER_TILE)
    out[ROWS_PER_TILE : (T - 1) * ROWS_PER_TILE] = (
        a[:, J : (T - 1) * J].reshape(P, T - 2, J).transpose(1, 0, 2).reshape(-1)
    )
    out[(T - 1) * ROWS_PER_TILE :] = a[:, (T - 1) * J :].T.reshape(ROWS_PER_TILE)
    return out


def kernel(logits: np.ndarray, labels: np.ndarray) -> np.ndarray:
    global LAST_RESULT
    logits = np.ascontiguousarray(logits, dtype=np.float32)
    labels_i = np.asarray(labels).astype(np.int64)

    nc = _get_nc()
    in_maps = [
        {"logits": logits[i * N_LOC : (i + 1) * N_LOC]} for i in range(N_CORES)
    ]
    res = run_bass_kernel_spmd(
        nc,
        in_maps,
        core_ids=list(range(N_CORES)),
        trace=os.environ.get("KERNEL_TRACE", "") == "1",
    )
    LAST_RESULT = res
    outs = res.results

    # --- host reassembly (tiny) ---
    V = np.zeros(C, dtype=np.float64)
    s_glob = np.empty(N, dtype=np.float64)
    for i in range(N_CORES):
        V += np.asarray(outs[i]["v_out"]).reshape(C).astype(np.float64)
        sl = slice(i * N_LOC, (i + 1) * N_LOC)
        s_glob[sl] = _unscramble(np.asarray(outs[i]["s_out"]).astype(np.float64))

    sumC = np.zeros((NB, C), dtype=np.float64)
    sumA = np.zeros((NB, C), dtype=np.float64)

    # accuracy side: only conf[n, labels[n]] matters
    lg_label = logits[np.arange(N), labels_i].astype(np.float64)
    conf_label = np.exp(lg_label) / s_glob
    valid = conf_label > 0.0
    bl = np.clip(np.ceil(conf_label * NB).astype(np.int64) - 1, 0, NB - 1)
    np.add.at(sumA, (bl[valid], labels_i[valid]), 1.0)

    # confidence side: everything starts in bin 0 via V; move the rare
    # elements with conf > 1/15 into their true bins (exact host recompute).
    # max conf per row = exp(rowmax) / s; rowmax is a cheap host pass.
    maxconf = np.exp(logits.max(axis=1).astype(np.float64)) / s_glob
    flagged = np.nonzero(maxconf > (1.0 / NB) * 0.98)[0]
    if flagged.size:
        xr = logits[flagged].astype(np.float64)
        er = np.exp(xr - xr.max(axis=1, keepdims=True))
        cr = er / er.sum(axis=1, keepdims=True)
        rows, cols = np.nonzero(cr > 1.0 / NB)
        if rows.size:
            vals = cr[rows, cols]
            bins = np.clip(np.ceil(vals * NB).astype(np.int64) - 1, 0, NB - 1)
            np.add.at(sumC, (bins, cols), vals)
            np.subtract.at(V, cols, vals)
    sumC[0] += V

    ece = np.abs(sumC - sumA).sum() / (N * C)
    return np.array([ece], dtype=np.float32)


# revision 20
# speedup vs baseline: 1.0145x; 1.0145x over previous
"""ECE (expected calibration error) kernel for Trainium2, 8 NeuronCores.

Math: per_bin = |avg_conf - avg_acc| * counts/N  ==  |sum_conf - sum_acc| / N
(when counts>0; both sides 0 when counts==0), so

    ECE = (1/(N*C)) * sum_{b,c} | sum_conf[b,c] - sum_acc[b,c] |

The device computes the heavy O(N*C) part per core (data-parallel over N):
  - V[c]     = sum_n conf[n,c]        (softmax column sums, PE-accumulated)
  - s[n]     = sum_c exp(logits[n,c]) (unshifted; logits bounded, no overflow)
  - max_e[n] = max_c exp(logits[n,c]) (so host can flag rows near bin edges)
The host assembles the per-(bin,class) sums from these:
  - bin 0 holds every element with conf <= 1/15; V gives its sum_conf column
    totals directly.  Rows whose max confidence max_e/s can reach 1/15 are
    recomputed exactly on host (a handful of rows) and their >1/15 elements
    are moved from bin 0 into their true bins.
  - sum_acc needs only conf[n, labels[n]] = exp(logits[n,labels[n]]) / s[n].

Device layout: quad-row tiles [128, 4, 1000] where partition r holds DRAM
rows 4r..4r+3 of the 512-row block -> 16KB contiguous per partition per DMA
descriptor; one 4000-element ACT exp op per tile; bf16 4x-mode Vector
reductions; per-row 1/s folded into the PE column-sum as the stationary.
"""

import os
import sys

import numpy as np

if "/opt/trn_rl_repo" not in sys.path:  # harness may run from a bare dir
    sys.path.insert(0, "/opt/trn_rl_repo")

import concourse.bass as bass
import concourse.tile as tile
from concourse import bacc, mybir
from concourse.bass_utils import run_bass_kernel_spmd

N, C, NB = 65536, 1000, 15
N_CORES = 8
N_LOC = N // N_CORES  # 8192
P = 128
J = 4  # rows per partition per tile
ROWS_PER_TILE = P * J  # 512
T = N_LOC // ROWS_PER_TILE  # 16 tiles per core
NCOL = T * J  # 64 stat columns
F32 = mybir.dt.float32
BF16 = mybir.dt.bfloat16

_CACHE: dict = {}
LAST_RESULT = None  # BassKernelResults of the most recent run (for profiling)


def _build():
    nc = bacc.Bacc("TRN2", target_bir_lowering=False, debug=False, num_devices=N_CORES)

    logits_ext = nc.declare_dram_parameter("logits", [N_LOC, C], F32, isOutput=False)
    v_ext = nc.declare_dram_parameter("v_out", [1, C], F32, isOutput=True)
    s_ext = nc.declare_dram_parameter("s_out", [P, NCOL], F32, isOutput=True)

    NA = 500  # first PSUM bank width
    NB_ = C - NA  # second

    with tile.TileContext(nc) as tc:
        with (
            tc.tile_pool(name="xin", bufs=5) as x_pool,
            tc.tile_pool(name="ework", bufs=4) as e_pool,
            tc.tile_pool(name="small", bufs=4) as w_pool,
            tc.tile_pool(name="accum", bufs=1) as acc_pool,
            tc.tile_pool(name="psum", bufs=1, space="PSUM") as psum_pool,
        ):
            s_acc = acc_pool.tile([P, NCOL], F32)
            pA = psum_pool.tile([1, NA], F32)
            pB = psum_pool.tile([1, NB_], F32)

            for t in range(T):
                x = x_pool.tile([P, J, C], F32, tag="x")
                src = logits_ext[
                    t * ROWS_PER_TILE : (t + 1) * ROWS_PER_TILE, :
                ].rearrange("(p j) c -> p j c", j=J)
                nc.sync.dma_start(out=x[:], in_=src)

                e = e_pool.tile([P, J, C], BF16, tag="e")
                nc.scalar.activation(e[:], x[:], mybir.ActivationFunctionType.Exp)

                nc.vector.tensor_reduce(
                    s_acc[:, t * J : (t + 1) * J],
                    e[:],
                    axis=mybir.AxisListType.X,
                    op=mybir.AluOpType.add,
                )

                w32 = w_pool.tile([P, J], F32, tag="w32")
                nc.vector.reciprocal(w32[:], s_acc[:, t * J : (t + 1) * J])
                w16 = w_pool.tile([P, J], BF16, tag="w16")
                nc.vector.tensor_copy(w16[:], w32[:])

                for j in range(J):
                    first = t == 0 and j == 0
                    last = t == T - 1 and j == J - 1
                    nc.tensor.matmul(
                        pA[:], w16[:, j : j + 1], e[:, j, :NA], start=first, stop=last
                    )
                    nc.tensor.matmul(
                        pB[:], w16[:, j : j + 1], e[:, j, NA:], start=first, stop=last
                    )

            vout = acc_pool.tile([1, C], F32)
            nc.scalar.copy(vout[:, :NA], pA[:])
            nc.scalar.copy(vout[:, NA:], pB[:])
            nc.sync.dma_start(out=v_ext[:], in_=vout[:])
            nc.sync.dma_start(out=s_ext[:], in_=s_acc[:])

    nc.compile()
    return nc


def _get_nc():
    if "nc" not in _CACHE:
        _CACHE["nc"] = _build()
    return _CACHE["nc"]


def _unscramble(a: np.ndarray) -> np.ndarray:
    # a[r, t*J + j] holds row t*ROWS_PER_TILE + r*J + j of the core's shard
    return a.reshape(P, T, J).transpose(1, 0, 2).reshape(N_LOC)


def kernel(logits: np.ndarray, labels: np.ndarray) -> np.ndarray:
    global LAST_RESULT
    logits = np.ascontiguousarray(logits, dtype=np.float32)
    labels_i = np.asarray(labels).astype(np.int64)

    nc = _get_nc()
    in_maps = [
        {"logits": logits[i * N_LOC : (i + 1) * N_LOC]} for i in range(N_CORES)
    ]
    res = run_bass_kernel_spmd(
        nc,
        in_maps,
        core_ids=list(range(N_CORES)),
        trace=os.environ.get("KERNEL_TRACE", "") == "1",
    )
    LAST_RESULT = res
    outs = res.results

    # --- host reassembly (tiny) ---
    V = np.zeros(C, dtype=np.float64)
    s_glob = np.empty(N, dtype=np.float64)
    for i in range(N_CORES):
        V += np.asarray(outs[i]["v_out"]).reshape(C).astype(np.float64)
        sl = slice(i * N_LOC, (i + 1) * N_LOC)
        s_glob[sl] = _unscramble(np.asarray(outs[i]["s_out"]).astype(np.float64))

    sumC = np.zeros((NB, C), dtype=np.float64)
    sumA = np.zeros((NB, C), dtype=np.float64)

    # accuracy side: only conf[n, labels[n]] matters
    lg_label = logits[np.arange(N), labels_i].astype(np.float64)
    conf_label = np.exp(lg_label) / s_glob
    valid = conf_label > 0.0
    bl = np.clip(np.ceil(conf_label * NB).astype(np.int64) - 1, 0, NB - 1)
    np.add.at(sumA, (bl[valid], labels_i[valid]), 1.0)

    # confidence side: everything starts in bin 0 via V; move the rare
    # elements with conf > 1/15 into their true bins (exact host recompute).
    # max conf per row = exp(rowmax) / s; rowmax is a cheap host pass.
    maxconf = np.exp(logits.max(axis=1).astype(np.float64)) / s_glob
    flagged = np.nonzero(maxconf > (1.0 / NB) * 0.98)[0]
    if flagged.size:
        xr = logits[flagged].astype(np.float64)
        er = np.exp(xr - xr.max(axis=1, keepdims=True))
        cr = er / er.sum(axis=1, keepdims=True)
        rows, cols = np.nonzero(cr > 1.0 / NB)
        if rows.size:
            vals = cr[rows, cols]
            bins = np.clip(np.ceil(vals * NB).astype(np.int64) - 1, 0, NB - 1)
            np.add.at(sumC, (bins, cols), vals)
            np.subtract.at(V, cols, vals)
    sumC[0] += V

    ece = np.abs(sumC - sumA).sum() / (N * C)
    return np.array([ece], dtype=np.float32)


# revision 22
# speedup vs baseline: 1.1839x; 1.1670x over previous
"""ECE (expected calibration error) kernel for Trainium2, 8 NeuronCores.

Math: per_bin = |avg_conf - avg_acc| * counts/N  ==  |sum_conf - sum_acc| / N
(when counts>0; both sides 0 when counts==0), so

    ECE = (1/(N*C)) * sum_{b,c} | sum_conf[b,c] - sum_acc[b,c] |

The device computes the heavy O(N*C) part per core (data-parallel over N):
  - V[c]     = sum_n conf[n,c]        (softmax column sums, PE-accumulated)
  - s[n]     = sum_c exp(logits[n,c]) (unshifted; logits bounded, no overflow)
  - max_e[n] = max_c exp(logits[n,c]) (so host can flag rows near bin edges)
The host assembles the per-(bin,class) sums from these:
  - bin 0 holds every element with conf <= 1/15; V gives its sum_conf column
    totals directly.  Rows whose max confidence max_e/s can reach 1/15 are
    recomputed exactly on host (a handful of rows) and their >1/15 elements
    are moved from bin 0 into their true bins.
  - sum_acc needs only conf[n, labels[n]] = exp(logits[n,labels[n]]) / s[n].

Device layout: quad-row tiles [128, 4, 1000] where partition r holds DRAM
rows 4r..4r+3 of the 512-row block -> 16KB contiguous per partition per DMA
descriptor; one 4000-element ACT exp op per tile; bf16 4x-mode Vector
reductions; per-row 1/s folded into the PE column-sum as the stationary.
"""

import os
import sys

import numpy as np

if "/opt/trn_rl_repo" not in sys.path:  # harness may run from a bare dir
    sys.path.insert(0, "/opt/trn_rl_repo")

import concourse.bass as bass
import concourse.tile as tile
from concourse import bacc, mybir
from concourse.bass_utils import run_bass_kernel_spmd

N, C, NB = 65536, 1000, 15
N_CORES = 8
N_LOC = N // N_CORES  # 8192
P = 128
J = 4  # rows per partition per tile
ROWS_PER_TILE = P * J  # 512
T = N_LOC // ROWS_PER_TILE  # 16 tiles per core
NCOL = T * J  # 64 stat columns
F32 = mybir.dt.float32
BF16 = mybir.dt.bfloat16

_CACHE: dict = {}
LAST_RESULT = None  # BassKernelResults of the most recent run (for profiling)


def _build():
    nc = bacc.Bacc("TRN2", target_bir_lowering=False, debug=False, num_devices=N_CORES)

    logits_ext = nc.declare_dram_parameter("logits", [N_LOC, C], F32, isOutput=False)
    v_ext = nc.declare_dram_parameter("v_out", [1, C], F32, isOutput=True)
    s_ext = nc.declare_dram_parameter("s_out", [P, NCOL], F32, isOutput=True)

    NA = 500  # first PSUM bank width
    NB_ = C - NA  # second

    with tile.TileContext(nc) as tc:
        with (
            tc.tile_pool(name="xin", bufs=5) as x_pool,
            tc.tile_pool(name="ework", bufs=4) as e_pool,
            tc.tile_pool(name="small", bufs=4) as w_pool,
            tc.tile_pool(name="accum", bufs=1) as acc_pool,
            tc.tile_pool(name="psum", bufs=1, space="PSUM") as psum_pool,
        ):
            s_acc = acc_pool.tile([P, NCOL], F32)
            pA = psum_pool.tile([1, NA], F32)
            pB = psum_pool.tile([1, NB_], F32)

            # The first 512-row block runs as four J=1 sub-units so the
            # pipeline ramps up after a 512KB DMA instead of a 2MB one.
            # Work items: (dram_row_start, rows_per_partition, s_col_start)
            work = [(k * P, 1, k) for k in range(J)]
            work += [(t * ROWS_PER_TILE, J, t * J) for t in range(1, T)]

            n_items = len(work)
            for it, (row0, jj, col0) in enumerate(work):
                x = x_pool.tile([P, jj, C], F32, tag=f"x{jj}")
                src = logits_ext[row0 : row0 + P * jj, :].rearrange(
                    "(p j) c -> p j c", j=jj
                )
                nc.sync.dma_start(out=x[:], in_=src)

                e = e_pool.tile([P, jj, C], BF16, tag=f"e{jj}")
                nc.scalar.activation(e[:], x[:], mybir.ActivationFunctionType.Exp)

                nc.vector.tensor_reduce(
                    s_acc[:, col0 : col0 + jj],
                    e[:],
                    axis=mybir.AxisListType.X,
                    op=mybir.AluOpType.add,
                )

                w32 = w_pool.tile([P, jj], F32, tag=f"w32{jj}")
                nc.vector.reciprocal(w32[:], s_acc[:, col0 : col0 + jj])
                w16 = w_pool.tile([P, jj], BF16, tag=f"w16{jj}")
                nc.vector.tensor_copy(w16[:], w32[:])

                for j in range(jj):
                    first = it == 0 and j == 0
                    last = it == n_items - 1 and j == jj - 1
                    nc.tensor.matmul(
                        pA[:], w16[:, j : j + 1], e[:, j, :NA], start=first, stop=last
                    )
                    nc.tensor.matmul(
                        pB[:], w16[:, j : j + 1], e[:, j, NA:], start=first, stop=last
                    )

            vout = acc_pool.tile([1, C], F32)
            nc.scalar.copy(vout[:, :NA], pA[:])
            nc.scalar.copy(vout[:, NA:], pB[:])
            nc.sync.dma_start(out=v_ext[:], in_=vout[:])
            nc.sync.dma_start(out=s_ext[:], in_=s_acc[:])

    nc.compile()
    return nc


def _get_nc():
    if "nc" not in _CACHE:
        _CACHE["nc"] = _build()
    return _CACHE["nc"]


def _unscramble(a: np.ndarray) -> np.ndarray:
    # Block 0 ran as four J=1 units: a[r, k] holds row k*P + r.
    # Quad blocks t>=1: a[r, t*J + j] holds row t*ROWS_PER_TILE + r*J + j.
    out = np.empty(N_LOC, dtype=a.dtype)
    out[:ROWS_PER_TILE] = a[:, :J].T.reshape(ROWS_PER_TILE)
    out[ROWS_PER_TILE:] = (
        a[:, J:].reshape(P, T - 1, J).transpose(1, 0, 2).reshape(-1)
    )
    return out


def kernel(logits: np.ndarray, labels: np.ndarray) -> np.ndarray:
    global LAST_RESULT
    logits = np.ascontiguousarray(logits, dtype=np.float32)
    labels_i = np.asarray(labels).astype(np.int64)

    nc = _get_nc()
    in_maps = [
        {"logits": logits[i * N_LOC : (i + 1) * N_LOC]} for i in range(N_CORES)
    ]
    res = run_bass_kernel_spmd(
        nc,
        in_maps,
        core_ids=list(range(N_CORES)),
        trace=os.environ.get("KERNEL_TRACE", "") == "1",
    )
    LAST_RESULT = res
    outs = res.results

    # --- host reassembly (tiny) ---
    V = np.zeros(C, dtype=np.float64)
    s_glob = np.empty(N, dtype=np.float64)
    for i in range(N_CORES):
        V += np.asarray(outs[i]["v_out"]).reshape(C).astype(np.float64)
        sl = slice(i * N_LOC, (i + 1) * N_LOC)
        s_glob[sl] = _unscramble(np.asarray(outs[i]["s_out"]).astype(np.float64))

    sumC = np.zeros((NB, C), dtype=np.float64)
    sumA = np.zeros((NB, C), dtype=np.float64)

    # accuracy side: only conf[n, labels[n]] matters
    lg_label = logits[np.arange(N), labels_i].astype(np.float64)
    conf_label = np.exp(lg_label) / s_glob
    valid = conf_label > 0.0
    bl = np.clip(np.ceil(conf_label * NB).astype(np.int64) - 1, 0, NB - 1)
    np.add.at(sumA, (bl[valid], labels_i[valid]), 1.0)

    # confidence side: everything starts in bin 0 via V; move the rare
    # elements with conf > 1/15 into their true bins (exact host recompute).
    # max conf per row = exp(rowmax) / s; rowmax is a cheap host pass.
    maxconf = np.exp(logits.max(axis=1).astype(np.float64)) / s_glob
    flagged = np.nonzero(maxconf > (1.0 / NB) * 0.98)[0]
    if flagged.size:
        xr = logits[flagged].astype(np.float64)
        er = np.exp(xr - xr.max(axis=1, keepdims=True))
        cr = er / er.sum(axis=1, keepdims=True)
        rows, cols = np.nonzero(cr > 1.0 / NB)
        if rows.size:
            vals = cr[rows, cols]
            bins = np.clip(np.ceil(vals * NB).astype(np.int64) - 1, 0, NB - 1)
            np.add.at(sumC, (bins, cols), vals)
            np.subtract.at(V, cols, vals)
    sumC[0] += V

    ece = np.abs(sumC - sumA).sum() / (N * C)
    return np.array([ece], dtype=np.float32)


# revision 26
# speedup vs baseline: 1.2281x; 1.0373x over previous
"""ECE (expected calibration error) kernel for Trainium2, 8 NeuronCores.

Math: per_bin = |avg_conf - avg_acc| * counts/N  ==  |sum_conf - sum_acc| / N
(when counts>0; both sides 0 when counts==0), so

    ECE = (1/(N*C)) * sum_{b,c} | sum_conf[b,c] - sum_acc[b,c] |

The device computes the heavy O(N*C) part per core (data-parallel over N):
  - V[c]     = sum_n conf[n,c]        (softmax column sums, PE-accumulated)
  - s[n]     = sum_c exp(logits[n,c]) (unshifted; logits bounded, no overflow)
  - max_e[n] = max_c exp(logits[n,c]) (so host can flag rows near bin edges)
The host assembles the per-(bin,class) sums from these:
  - bin 0 holds every element with conf <= 1/15; V gives its sum_conf column
    totals directly.  Rows whose max confidence max_e/s can reach 1/15 are
    recomputed exactly on host (a handful of rows) and their >1/15 elements
    are moved from bin 0 into their true bins.
  - sum_acc needs only conf[n, labels[n]] = exp(logits[n,labels[n]]) / s[n].

Device layout: quad-row tiles [128, 4, 1000] where partition r holds DRAM
rows 4r..4r+3 of the 512-row block -> 16KB contiguous per partition per DMA
descriptor; one 4000-element ACT exp op per tile; bf16 4x-mode Vector
reductions; per-row 1/s folded into the PE column-sum as the stationary.
"""

import os
import sys

import numpy as np

if "/opt/trn_rl_repo" not in sys.path:  # harness may run from a bare dir
    sys.path.insert(0, "/opt/trn_rl_repo")

import concourse.bass as bass
import concourse.tile as tile
from concourse import bacc, mybir
from concourse.bass_utils import run_bass_kernel_spmd

N, C, NB = 65536, 1000, 15
N_CORES = 8
N_LOC = N // N_CORES  # 8192
P = 128
J = 4  # rows per partition per tile
ROWS_PER_TILE = P * J  # 512
T = N_LOC // ROWS_PER_TILE  # 16 tiles per core
NCOL = T * J  # 64 stat columns
F32 = mybir.dt.float32
BF16 = mybir.dt.bfloat16

_CACHE: dict = {}
LAST_RESULT = None  # BassKernelResults of the most recent run (for profiling)


def _build():
    nc = bacc.Bacc("TRN2", target_bir_lowering=False, debug=False, num_devices=N_CORES)

    logits_ext = nc.declare_dram_parameter("logits", [N_LOC, C], F32, isOutput=False)
    v_ext = nc.declare_dram_parameter("v_out", [1, C], F32, isOutput=True)
    s_ext = nc.declare_dram_parameter("s_out", [P, NCOL], F32, isOutput=True)

    NA = 500  # first PSUM bank width
    NB_ = C - NA  # second

    with tile.TileContext(nc) as tc:
        with (
            tc.tile_pool(name="xin", bufs=5) as x_pool,
            tc.tile_pool(name="ework", bufs=4) as e_pool,
            tc.tile_pool(name="small", bufs=4) as w_pool,
            tc.tile_pool(name="accum", bufs=1) as acc_pool,
            tc.tile_pool(name="psum", bufs=1, space="PSUM") as psum_pool,
        ):
            s_acc = acc_pool.tile([P, NCOL], F32)
            pA = psum_pool.tile([1, NA], F32)
            pB = psum_pool.tile([1, NB_], F32)

            # The first and last 512-row blocks run as four J=1 sub-units:
            # the pipeline ramps up after a 512KB DMA instead of a 2MB one,
            # and the tail chain after the final DMA holds one 1.5us
            # exp+accum instead of four.
            # Work items: (dram_row_start, rows_per_partition, s_col_start)
            work = [(k * P, 1, k) for k in range(J)]
            work += [(t * ROWS_PER_TILE, J, t * J) for t in range(1, T - 1)]
            work += [
                ((T - 1) * ROWS_PER_TILE + k * P, 1, (T - 1) * J + k)
                for k in range(J)
            ]

            n_items = len(work)
            for it, (row0, jj, col0) in enumerate(work):
                x = x_pool.tile([P, jj, C], F32, tag=f"x{jj}")
                src = logits_ext[row0 : row0 + P * jj, :].rearrange(
                    "(p j) c -> p j c", j=jj
                )
                nc.sync.dma_start(out=x[:], in_=src)

                e = e_pool.tile([P, jj, C], BF16, tag=f"e{jj}")
                if it >= n_items - 6:
                    # Last tiles: rowsum via Scalar's activation accumulator
                    # (per-row exp ops) so Vector drains early and the tail
                    # chain skips its 4.3us reduce.  Balances Scalar ~64us
                    # vs Vector ~61us, both well under DMA's ~78us.
                    for j in range(jj):
                        nc.scalar.activation(
                            e[:, j, :],
                            x[:, j, :],
                            mybir.ActivationFunctionType.Exp,
                            accum_out=s_acc[:, col0 + j : col0 + j + 1],
                        )
                else:
                    nc.scalar.activation(
                        e[:], x[:], mybir.ActivationFunctionType.Exp
                    )
                    nc.vector.tensor_reduce(
                        s_acc[:, col0 : col0 + jj],
                        e[:],
                        axis=mybir.AxisListType.X,
                        op=mybir.AluOpType.add,
                    )

                w32 = w_pool.tile([P, jj], F32, tag=f"w32{jj}")
                nc.vector.reciprocal(w32[:], s_acc[:, col0 : col0 + jj])
                w16 = w_pool.tile([P, jj], BF16, tag=f"w16{jj}")
                nc.vector.tensor_copy(w16[:], w32[:])

                for j in range(jj):
                    first = it == 0 and j == 0
                    last = it == n_items - 1 and j == jj - 1
                    nc.tensor.matmul(
                        pA[:], w16[:, j : j + 1], e[:, j, :NA], start=first, stop=last
                    )
                    nc.tensor.matmul(
                        pB[:], w16[:, j : j + 1], e[:, j, NA:], start=first, stop=last
                    )

            vout = acc_pool.tile([1, C], F32)
            nc.scalar.copy(vout[:, :NA], pA[:])
            nc.scalar.copy(vout[:, NA:], pB[:])
            nc.sync.dma_start(out=v_ext[:], in_=vout[:])
            nc.sync.dma_start(out=s_ext[:], in_=s_acc[:])

    nc.compile()
    return nc


def _get_nc():
    if "nc" not in _CACHE:
        _CACHE["nc"] = _build()
    return _CACHE["nc"]


def _unscramble(a: np.ndarray) -> np.ndarray:
    # Blocks 0 and T-1 ran as four J=1 units: a[r, t*J + k] holds row
    # t*ROWS_PER_TILE + k*P + r.  Quad blocks 1..T-2: a[r, t*J + j] holds
    # row t*ROWS_PER_TILE + r*J + j.
    out = np.empty(N_LOC, dtype=a.dtype)
    out[:ROWS_PER_TILE] = a[:, :J].T.reshape(ROWS_PER_TILE)
    out[ROWS_PER_TILE : (T - 1) * ROWS_PER_TILE] = (
        a[:, J : (T - 1) * J].reshape(P, T - 2, J).transpose(1, 0, 2).reshape(-1)
    )
    out[(T - 1) * ROWS_PER_TILE :] = a[:, (T - 1) * J :].T.reshape(ROWS_PER_TILE)
    return out


def kernel(logits: np.ndarray, labels: np.ndarray) -> np.ndarray:
    global LAST_RESULT
    logits = np.ascontiguousarray(logits, dtype=np.float32)
    labels_i = np.asarray(labels).astype(np.int64)

    nc = _get_nc()
    in_maps = [
        {"logits": logits[i * N_LOC : (i + 1) * N_LOC]} for i in range(N_CORES)
    ]
    res = run_bass_kernel_spmd(
        nc,
        in_maps,
        core_ids=list(range(N_CORES)),
        trace=os.environ.get("KERNEL_TRACE", "") == "1",
    )
    LAST_RESULT = res
    outs = res.results

    # --- host reassembly (tiny) ---
    V = np.zeros(C, dtype=np.float64)
    s_glob = np.empty(N, dtype=np.float64)
    for i in range(N_CORES):
        V += np.asarray(outs[i]["v_out"]).reshape(C).astype(np.float64)
        sl = slice(i * N_LOC, (i + 1) * N_LOC)
        s_glob[sl] = _unscramble(np.asarray(outs[i]["s_out"]).astype(np.float64))

    sumC = np.zeros((NB, C), dtype=np.float64)
    sumA = np.zeros((NB, C), dtype=np.float64)

    # accuracy side: only conf[n, labels[n]] matters
    lg_label = logits[np.arange(N), labels_i].astype(np.float64)
    conf_label = np.exp(lg_label) / s_glob
    valid = conf_label > 0.0
    bl = np.clip(np.ceil(conf_label * NB).astype(np.int64) - 1, 0, NB - 1)
    np.add.at(sumA, (bl[valid], labels_i[valid]), 1.0)

    # confidence side: everything starts in bin 0 via V; move the rare
    # elements with conf > 1/15 into their true bins (exact host recompute).
    # max conf per row = exp(rowmax) / s; rowmax is a cheap host pass.
    maxconf = np.exp(logits.max(axis=1).astype(np.float64)) / s_glob
    flagged = np.nonzero(maxconf > (1.0 / NB) * 0.98)[0]
    if flagged.size:
        xr = logits[flagged].astype(np.float64)
        er = np.exp(xr - xr.max(axis=1, keepdims=True))
        cr = er / er.sum(axis=1, keepdims=True)
        rows, cols = np.nonzero(cr > 1.0 / NB)
        if rows.size:
            vals = cr[rows, cols]
            bins = np.clip(np.ceil(vals * NB).astype(np.int64) - 1, 0, NB - 1)
            np.add.at(sumC, (bins, cols), vals)
            np.subtract.at(V, cols, vals)
    sumC[0] += V

    ece = np.abs(sumC - sumA).sum() / (N * C)
    return np.array([ece], dtype=np.float32)
